# revision 1
# baseline (speedup 1.0000x reference)
"""Trainium2 Bass kernel for nn_DWTExtractor: 2-level Haar DWT + bilinear 2x upsample.

Input  x: (32, 1, 1024, 1024) fp32
Output y: (32, 6, 512, 512) fp32 = [cH1, cV1, cD1, cH2u, cV2u, cD2u]

Sharding: pure batch data-parallel, 4 images per core across 8 cores.

Per-core dataflow (per image, 4 chunks of 256 input rows):
  - PE (fp32r) computes all cross-row (H-direction) work as banded matmuls:
      L1/L2 Haar row-pairing (+-0.5 weights baked in) and the bilinear
      H-upsample (0.75/0.25 taps, x0.25 de-scale folded in).
  - ACT evacuates even-index columns of PSUM (strided copies).
  - DVE does the W-direction pair sum/diff as tensor_tensor with one SBUF
    (evacuated evens) and one strided PSUM (odds) operand.
  - GPSIMD does the W-direction bilinear upsample: t3 = 3*band, then
    out_even = t3 + band[j-1], out_odd = t3 + band[j+1] (values are 4x the
    true upsample; the 1/4 is folded into the H-upsample matrices).
"""

import numpy as np

import concourse.bass as bass
import concourse.tile as tile
import concourse.mybir as mybir
from concourse import bacc, bass_utils

F32 = mybir.dt.float32
F32R = mybir.dt.float32r
AL = mybir.AluOpType

B, H, W = 32, 1024, 1024
NCORES = 8
IMG = B // NCORES  # images per core
HL, WL = H // 2, W // 2  # 512, 512 (level-1 band size)
H2, W2 = H // 4, W // 4  # 256, 256 (level-2 band size)
P = 128


def _build_const_matrix() -> np.ndarray:
    """(128, 10*128) fp32: PS_lo|PS_hi|PD_lo|PD_hi|U0|U1|U2|U3|U1b|U2b."""
    ps_lo = np.zeros((P, P), np.float32)
    ps_hi = np.zeros((P, P), np.float32)
    pd_lo = np.zeros((P, P), np.float32)
    pd_hi = np.zeros((P, P), np.float32)
    for i in range(64):
        ps_lo[2 * i, i] = 0.5
        ps_lo[2 * i + 1, i] = 0.5
        ps_hi[2 * i, 64 + i] = 0.5
        ps_hi[2 * i + 1, 64 + i] = 0.5
        pd_lo[2 * i, i] = 0.5
        pd_lo[2 * i + 1, i] = -0.5
        pd_hi[2 * i, 64 + i] = 0.5
        pd_hi[2 * i + 1, 64 + i] = -0.5

    # H-upsample matrix (256 src rows -> 512 out rows), half-pixel bilinear
    # with edge clamp; x0.25 folded in (wup values are 4x true).
    u_full = np.zeros((H2, HL), np.float32)
    for m in range(HL):
        k = m // 2
        if m % 2 == 0:
            taps = [(k, 0.75), (k - 1, 0.25)]
        else:
            taps = [(k, 0.75), (k + 1, 0.25)]
        for src, wgt in taps:
            u_full[min(max(src, 0), H2 - 1), m] += wgt
    u_full *= 0.25

    u0 = u_full[0:128, 0:128]
    u1 = u_full[0:128, 128:256]
    u2 = u_full[128:256, 256:384]
    u3 = u_full[128:256, 384:512]
    u1b = np.zeros((P, P), np.float32)
    u1b[0, :] = u_full[128, 128:256]
    u2b = np.zeros((P, P), np.float32)
    u2b[127, :] = u_full[127, 256:384]

    return np.concatenate(
        [ps_lo, ps_hi, pd_lo, pd_hi, u0, u1, u2, u3, u1b, u2b], axis=1
    )


def build_nc() -> "bacc.Bacc":
    nc = bacc.Bacc(
        "TRN2", target_bir_lowering=False, debug=False, num_devices=NCORES,
        name="dwt_extractor",
    )
    x_d = nc.dram_tensor("xc", [IMG, H, W], F32R, kind="ExternalInput")
    wm_d = nc.dram_tensor("wm", [P, 10 * P], F32R, kind="ExternalInput")
    y_d = nc.dram_tensor("yc", [IMG, 6, HL, WL], F32, kind="ExternalOutput")

    with tile.TileContext(nc) as tc:
        with (
            tc.tile_pool(name="consts", bufs=1) as cpool,
            tc.tile_pool(name="xin", bufs=5) as xpool,
            tc.tile_pool(name="ev", bufs=7) as evpool,
            tc.tile_pool(name="bands3", bufs=4) as b3pool,
            tc.tile_pool(name="t3", bufs=3) as t3pool,
            tc.tile_pool(name="wup3", bufs=4) as wuppool,
            tc.tile_pool(name="stg", bufs=2) as stgpool,
            tc.tile_pool(name="stg2", bufs=1) as stg2pool,
            tc.tile_pool(name="psS", bufs=1, space="PSUM") as psS,
            tc.tile_pool(name="psD", bufs=1, space="PSUM") as psD,
            tc.tile_pool(name="psL2", bufs=1, space="PSUM") as psL2,
            tc.tile_pool(name="psUp", bufs=2, space="PSUM") as psUp,
        ):
            wm = cpool.tile([P, 10 * P], F32R)
            nc.sync.dma_start(wm[:, 0 : 4 * P], wm_d[:, 0 : 4 * P])
            nc.sync.dma_start(wm[:, 4 * P :], wm_d[:, 4 * P :])
            blk = lambda i: wm[:, i * P : (i + 1) * P]
            PS_lo, PS_hi, PD_lo, PD_hi = blk(0), blk(1), blk(2), blk(3)
            U0, U1, U2, U3 = blk(4), blk(5), blk(6), blk(7)
            U1b = blk(8)
            U2b = blk(9)

            def stage_a(b, defer_bands=False):
                """L1 chunks + L2 + W-upsample for image b; returns wup3s."""
                ca1 = []
                stgL1 = []
                deferred = []
                for u in range(4):
                    xu = xpool.tile([P, 2048], F32R, tag="x")
                    src = x_d[b, 256 * u : 256 * (u + 1), :]
                    nc.sync.dma_start(
                        xu[:].rearrange("p (t w) -> p t w", t=2),
                        src.rearrange("(t p) w -> p t w", t=2),
                    )
                    if u == 0:
                        stgH1 = stgpool.tile([P, 2048], F32, tag="sH1")
                        stgV1 = stgpool.tile([P, 2048], F32, tag="sV1")
                        stgD1 = stgpool.tile([P, 2048], F32, tag="sD1")
                        stgL1 = [stgH1, stgV1, stgD1]
                    o512 = 512 * u
                    # half-granular L1 psum (finer slot release); both halves
                    # evacuate into ONE f32r sbuf tile; the L2 matmuls do the
                    # W-pairing themselves via strided rhs + psum accumulation
                    sf = evpool.tile([P, 1024], F32R, tag="sf")
                    ca1.append(sf)
                    for h in range(2):
                        o = 512 * h
                        sS = psS.tile([P, 512], F32, tag=f"S{h}")
                        nc.tensor.matmul(
                            sS[:], PS_lo, xu[:, o : o + 512],
                            start=True, stop=False,
                        )
                        nc.tensor.matmul(
                            sS[:], PS_hi, xu[:, 1024 + o : 1536 + o],
                            start=False, stop=True,
                        )
                        nc.scalar.copy(sf[:, o : o + 512], sS[:])
                    sf32 = sf[:].bitcast(F32)
                    deferred.append((stgL1[0], o512, sf32))
                    df = evpool.tile([P, 1024], F32, tag="sf")
                    for h in range(2):
                        o = 512 * h
                        sD = psD.tile([P, 512], F32, tag=f"D{h}")
                        nc.tensor.matmul(
                            sD[:], PD_lo, xu[:, o : o + 512],
                            start=True, stop=False,
                        )
                        nc.tensor.matmul(
                            sD[:], PD_hi, xu[:, 1024 + o : 1536 + o],
                            start=False, stop=True,
                        )
                        nc.scalar.copy(df[:, o : o + 512], sD[:])
                    deferred.append((stgL1[1], stgL1[2], o512, df))
                    if not defer_bands:
                        flush_bands(deferred)

                # level 2 + W-upsample; wup3s[v] = (128, 3*512) f32r
                wup3s = [None, None]
                for v in range(2):
                    s2 = psL2.tile([P, 512], F32, tag="s2")
                    d2 = psL2.tile([P, 512], F32, tag="d2")
                    r0 = ca1[2 * v][:]
                    r1 = ca1[2 * v + 1][:]
                    nc.tensor.matmul(s2[:], PS_lo, r0[:, 0:1024:2], start=True, stop=False)
                    nc.tensor.matmul(s2[:], PS_lo, r0[:, 1:1024:2], start=False, stop=False)
                    nc.tensor.matmul(s2[:], PS_hi, r1[:, 0:1024:2], start=False, stop=False)
                    nc.tensor.matmul(s2[:], PS_hi, r1[:, 1:1024:2], start=False, stop=True)
                    nc.tensor.matmul(d2[:], PD_lo, r0[:, 0:1024:2], start=True, stop=False)
                    nc.tensor.matmul(d2[:], PD_lo, r0[:, 1:1024:2], start=False, stop=False)
                    nc.tensor.matmul(d2[:], PD_hi, r1[:, 0:1024:2], start=False, stop=False)
                    nc.tensor.matmul(d2[:], PD_hi, r1[:, 1:1024:2], start=False, stop=True)

                    s2f = evpool.tile([P, 512], F32, tag="s2f")
                    d2f = evpool.tile([P, 512], F32, tag="s2f")
                    nc.scalar.copy(s2f[:], s2[:])
                    nc.scalar.copy(d2f[:], d2[:])

                    b3 = b3pool.tile([P, 768], F32, tag="b3")
                    nc.vector.tensor_tensor(
                        b3[:, 0:256], s2f[:, 0:512:2], s2f[:, 1:512:2], AL.subtract
                    )  # cH2
                    nc.vector.tensor_tensor(
                        b3[:, 256:512], d2f[:, 0:512:2], d2f[:, 1:512:2], AL.add
                    )  # cV2
                    # cD2 = 2*D2e - cV2
                    nc.vector.scalar_tensor_tensor(
                        b3[:, 512:768], d2f[:, 0:512:2], 2.0, b3[:, 256:512],
                        AL.mult, AL.subtract,
                    )
                    t3 = t3pool.tile([P, 768], F32, tag="t3")
                    nc.vector.tensor_scalar_mul(t3[:], b3[:], 3.0)
                    wu = wuppool.tile([P, 1536], F32R, tag="wup")
                    wu_r = wu[:].rearrange("p (b w) -> p b w", b=3)
                    b3_r = b3[:].rearrange("p (b w) -> p b w", b=3)
                    t3_r = t3[:].rearrange("p (b w) -> p b w", b=3)
                    nc.gpsimd.tensor_tensor(
                        wu_r[:, :, 2:512:2], t3_r[:, :, 1:256], b3_r[:, :, 0:255], AL.add
                    )
                    nc.gpsimd.tensor_tensor(
                        wu_r[:, :, 1:511:2], t3_r[:, :, 0:255], b3_r[:, :, 1:256], AL.add
                    )
                    nc.vector.tensor_scalar_mul(
                        wu_r[:, :, 0:512:511], b3_r[:, :, 0:256:255], 4.0
                    )
                    wup3s[v] = wu

                if defer_bands:
                    flush_bands(deferred)
                # L1 band outputs can stream out now
                for band in range(3):
                    dst = y_d[b, band]
                    nc.gpsimd.dma_start(
                        dst.rearrange("(u p) w -> p u w", u=4),
                        stgL1[band][:].rearrange("p (u w) -> p u w", u=4),
                    )
                return wup3s

            def stage_b(b, wup3s, split_outs=False, tail=False):
                """H-upsample + evacuation + upsampled-band outputs for image b."""
                ncopy = [0]

                def evac(dst_ap, src_ap):
                    ncopy[0] += 1
                    if tail and ncopy[0] % 2 == 0:
                        nc.vector.tensor_copy(dst_ap, src_ap)
                    else:
                        nc.scalar.copy(dst_ap, src_ap)

                for band in range(3):
                    w0 = wup3s[0][:, 512 * band : 512 * (band + 1)]
                    w1 = wup3s[1][:, 512 * band : 512 * (band + 1)]
                    st = stg2pool.tile([P, 2048], F32, tag=f"s2b{band}")
                    dst = y_d[b, 3 + band]
                    dst_r = dst.rearrange("(u p) w -> p u w", u=4)
                    st_r = st[:].rearrange("p (u w) -> p u w", u=4)
                    up = psUp.tile([P, 512], F32, tag="up")
                    nc.tensor.matmul(up[:], U0, w0, start=True, stop=True)
                    evac(st[:, 0:512], up[:])
                    up = psUp.tile([P, 512], F32, tag="up")
                    nc.tensor.matmul(up[:], U1, w0, start=True, stop=False)
                    nc.tensor.matmul(up[:], U1b, w1, start=False, stop=True)
                    evac(st[:, 512:1024], up[:])
                    up = psUp.tile([P, 512], F32, tag="up")
                    nc.tensor.matmul(up[:], U2, w1, start=True, stop=False)
                    nc.tensor.matmul(up[:], U2b, w0, start=False, stop=True)
                    evac(st[:, 1024:1536], up[:])
                    up = psUp.tile([P, 512], F32, tag="up")
                    nc.tensor.matmul(up[:], U3, w1, start=True, stop=True)
                    evac(st[:, 1536:2048], up[:])
                    if split_outs:
                        nc.sync.dma_start(dst_r[:, 0:2], st_r[:, 0:2])
                        nc.sync.dma_start(dst_r[:, 2:4], st_r[:, 2:4])
                    else:
                        nc.gpsimd.dma_start(dst_r, st_r)

            def flush_bands(deferred):
                while deferred:
                    item = deferred.pop(0)
                    if len(item) == 3:
                        stg0, o512, sf32 = item
                        nc.vector.tensor_tensor(
                            stg0[:, o512 : o512 + 512],
                            sf32[:, 0:1024:2], sf32[:, 1:1024:2], AL.subtract,
                        )
                    else:
                        stg1, stg2s, o512, df = item
                        nc.vector.tensor_tensor(
                            stg1[:, o512 : o512 + 512],
                            df[:, 0:1024:2], df[:, 1:1024:2], AL.add,
                        )
                        nc.vector.scalar_tensor_tensor(
                            stg2s[:, o512 : o512 + 512], df[:, 0:1024:2], 2.0,
                            stg1[:, o512 : o512 + 512], AL.mult, AL.subtract,
                        )

            pending = None
            for b in range(IMG):
                wup3s = stage_a(b, defer_bands=(b == IMG - 1))
                if pending is not None:
                    stage_b(pending[0], pending[1])
                pending = (b, wup3s)
            stage_b(pending[0], pending[1])

    nc.compile()
    return nc


_NC_CACHE = None
LAST_RESULTS = None


def kernel(**inputs) -> np.ndarray:
    global _NC_CACHE, LAST_RESULTS
    trace = bool(inputs.pop("_trace", False))
    x = np.ascontiguousarray(np.asarray(inputs["x"], dtype=np.float32))
    assert x.shape == (B, 1, H, W), x.shape
    if _NC_CACHE is None:
        _NC_CACHE = build_nc()
    nc = _NC_CACHE
    wm = _build_const_matrix()
    in_maps = [
        {"xc": np.ascontiguousarray(x[IMG * c : IMG * (c + 1), 0]), "wm": wm}
        for c in range(NCORES)
    ]
    res = bass_utils.run_bass_kernel_spmd(
        nc, in_maps, core_ids=list(range(NCORES)), trace=trace
    )
    LAST_RESULTS = res
    out = np.concatenate([res.results[c]["yc"] for c in range(NCORES)], axis=0)
    return out.astype(np.float32)


if __name__ == "__main__":
    rng = np.random.default_rng(0)
    x = rng.standard_normal((B, 1, H, W), dtype=np.float32)
    y = kernel(x=x)
    print("kernel output:", y.shape, y.dtype)

